# revision 1
# baseline (speedup 1.0000x reference)
"""DILATE loss (soft-DTW fwd + grad, gamma=0.01 ~ hard-min) on 8 TRN2 cores.

Batch-parallel: 8 samples/core. Per core, the 64 (sample, col-block) DP scans
run as a skewed wavefront: 4 col-blocks of 64 columns, block q on SBUF
quadrant q (lanes 32q+0..7). Slot t of block q holds DP row i = t - q in a
65-float record [chain | 64 cols]; tensor_tensor_scan computes each row's
min-plus recurrence in one instruction, with the cross-block chain value
injected as scan element 0 via quadrant-aligned copies. The soft-DTW gradient
is the hard argmin-mask linear recurrence run as a reversed scan; masks are
equality-derived in batched chunks and bounced through DRAM.
"""
import numpy as np
import ml_dtypes

bf16 = ml_dtypes.bfloat16
f32 = np.float32

ALPHA = 0.5
BIG = 1e8
B, N = 64, 256
Q, C = 4, 65
S, SE = 260, 262
NCORES = 8
SPC = B // NCORES
MCH = 8    # mask-phase chunk (slots)
WCH = 8    # backward mask window stride (slots); window covers WCH+1 slots

_cache = {}


def _build(repeat=1, phases="dfmbx", fwdops="cms"):
    import concourse.bacc as bacc
    import concourse.tile as tile
    import concourse.mybir as mybir
    from contextlib import ExitStack

    dt = mybir.dt
    Alu = mybir.AluOpType

    nc = bacc.Bacc("TRN2", target_bir_lowering=False, debug=False)
    dT_d = nc.dram_tensor("dT", [128, S], dt.float32, kind="ExternalInput").ap()
    dO_d = nc.dram_tensor("dO", [128, 64], dt.float32, kind="ExternalInput").ap()
    mx_d = nc.dram_tensor("mx", [128, S * C], dt.bfloat16, kind="ExternalInput").ap()
    ps_d = nc.dram_tensor("ps", [8, 1], dt.float32, kind="ExternalOutput").ap()
    pt_d = nc.dram_tensor("pt", [128, 1], dt.float32, kind="ExternalOutput").ap()
    mU_d = nc.dram_tensor("mU_s", [128, SE * C], dt.bfloat16).ap()
    mD_d = nc.dram_tensor("mD_s", [128, SE * C], dt.bfloat16).ap()
    mL_d = nc.dram_tensor("mL_s", [128, SE * C], dt.bfloat16).ap()

    with tile.TileContext(nc) as tc:
        with ExitStack() as ctx:
            big = ctx.enter_context(tc.tile_pool(name="big", bufs=1))
            st_pool = ctx.enter_context(tc.tile_pool(name="stage", bufs=2))
            win_pool = ctx.enter_context(tc.tile_pool(name="win", bufs=2))
            sc_pool = ctx.enter_context(tc.tile_pool(name="scr", bufs=2))

            h = big.tile([128, S * C], dt.float32, tag="h")
            d = big.tile([128, S * C], dt.bfloat16, tag="d")
            E = big.tile([128, SE * C], dt.float32, tag="E")
            dT = big.tile([128, S], dt.float32, tag="dT")
            dO = big.tile([128, 64], dt.float32, tag="dO")
            c0 = big.tile([128, C], dt.float32, tag="c0")
            c1 = big.tile([128, C], dt.float32, tag="c1")
            G0 = big.tile([128, 66], dt.float32, tag="G0")
            G1t = big.tile([128, 66], dt.float32, tag="G1t")
            S0 = big.tile([128, 66], dt.float32, tag="S0")
            S1t = big.tile([128, 66], dt.float32, tag="S1t")
            zb = big.tile([128, 2 * C], dt.bfloat16, tag="zb")
            pt_t = big.tile([128, 1], dt.float32, tag="pt_t")
            c_tiles = [c0, c1]
            G_tiles = [G0, G1t]
            S_tiles = [S0, S1t]

            # inputs
            nc.sync.dma_start(out=dT[:], in_=dT_d[:])
            nc.sync.dma_start(out=dO[:], in_=dO_d[:])
            for _rep in range(repeat):
                # E zero on gpsimd (runs concurrent with fwd on DVE)
                nc.gpsimd.memset(E[:], 0.0)
                nc.gpsimd.memset(zb[:], 0.0)

                # D build: d[p, t*C+1+jl] = (dT[p,t]-dO[p,jl])^2  (bf16)
                if "d" not in phases:
                    continue
                nc.vector.memset(d[:], 0.0)
                DCH = 33
                for k0 in range(0, S, DCH):
                    k1 = min(k0 + DCH, S)
                    d3 = d[:].rearrange("p (s c) -> p s c", c=C)[:, k0:k1, 1:]
                    nc.vector.tensor_tensor(
                        d3, dT[:, k0:k1].unsqueeze(2).broadcast_to([128, k1 - k0, 64]),
                        dO[:].unsqueeze(1).broadcast_to([128, k1 - k0, 64]), Alu.subtract)
                    nc.vector.tensor_tensor(d3, d3, d3, Alu.mult)

                # fwd prefills
                for q in range(Q):
                    nc.vector.memset(h[32 * q:32 * q + 32, q * C:(q + 1) * C], BIG)
                nc.vector.memset(h[0:8, 0:1], 0.0)
                for ct in c_tiles:
                    nc.vector.memset(ct[0:32, 0:1], BIG)
                for gt in G_tiles:
                    nc.vector.memset(gt[:, 0:1], 0.0)
                    nc.vector.memset(gt[96:128, 65:66], 0.0)

                # ---------------- forward ----------------
                if "f" not in phases:
                    continue
                def prange(qlo, qhi):
                    P0, P1 = 32 * qlo, 32 * qhi + 32
                    cnt = P1 - P0
                    if not (cnt <= 32 or P0 == 0 or (P0 == 64 and cnt <= 64)):
                        P0 = 0
                    return P0, P1

                for t in range(1, S):
                    qlo, qhi = max(0, t - 256), min(3, t - 1)
                    P0, P1 = prange(qlo, qhi)
                    ct = c_tiles[t % 2]
                    if "c" in fwdops:
                        for q in range(max(1, qlo), qhi + 1):
                            nc.gpsimd.tensor_copy(
                                ct[32 * q:32 * q + 32, 0:1],
                                h[32 * (q - 1):32 * q, (t - 1) * C + 64:(t - 1) * C + 65])
                    if "m" in fwdops:
                        nc.vector.tensor_tensor(
                            ct[P0:P1, 1:65],
                            h[P0:P1, (t - 1) * C + 1:(t - 1) * C + 65],
                            h[P0:P1, (t - 1) * C:(t - 1) * C + 64], Alu.min)
                    if "s" in fwdops:
                        # state = min(c'_j, state) + d_j  (c' excludes d; chain in c'[0])
                        nc.vector.tensor_tensor_scan(
                            h[P0:P1, t * C:t * C + 65],
                            ct[P0:P1, 0:65],
                            d[P0:P1, t * C:t * C + 65], float(BIG), Alu.min, Alu.add)

                # loss_shape partials
                nc.sync.dma_start(out=ps_d[:], in_=h[96:104, 259 * C + 64:259 * C + 65])

                # ---------------- mask phase ----------------
                if "m" not in phases:
                    continue
                for s0 in range(1, S, MCH):
                    s1 = min(s0 + MCH, S)
                    ns = s1 - s0
                    cX = sc_pool.tile([128, MCH * C], dt.float32, tag="cX")
                    mu = st_pool.tile([128, MCH * C], dt.bfloat16, tag="mu")
                    md = st_pool.tile([128, MCH * C], dt.bfloat16, tag="md")
                    ml = st_pool.tile([128, MCH * C], dt.bfloat16, tag="ml")
                    hv = h[:].rearrange("p (s c) -> p s c", c=C)
                    dv = d[:].rearrange("p (s c) -> p s c", c=C)
                    cXv = cX[:].rearrange("p (s c) -> p s c", c=C)[:, 0:ns, :]
                    for m_t, hoff in ((mu, hv[:, s0 - 1:s1 - 1, 1:]),
                                      (md, hv[:, s0 - 1:s1 - 1, 0:64]),
                                      (ml, hv[:, s0:s1, 0:64])):
                        nc.vector.tensor_tensor(cXv[:, :, 1:], dv[:, s0:s1, 1:], hoff, Alu.add)
                        mv = m_t[:].rearrange("p (s c) -> p s c", c=C)[:, 0:ns, :]
                        nc.vector.tensor_tensor(mv[:, :, 1:], hv[:, s0:s1, 1:],
                                                cXv[:, :, 1:], Alu.is_equal)
                    # margins on md, ml
                    for m_t in (md, ml):
                        mv = m_t[:].rearrange("p (s c) -> p s c", c=C)[:, 0:ns, :]
                        for q in (0, 1, 2):
                            nc.gpsimd.tensor_copy(
                                mv[32 * q:32 * q + 32, :, 0:1],
                                mv[32 * (q + 1):32 * (q + 1) + 32, :, 1:2])
                        nc.gpsimd.memset(mv[96:128, :, 0:1], 0.0)
                    for m_t, m_dram in ((mu, mU_d), (md, mD_d), (ml, mL_d)):
                        nc.sync.dma_start(out=m_dram[0:104, s0 * C:s1 * C],
                                          in_=m_t[0:104, 0:ns * C])
                # zero-fill DRAM mask slots 260..261
                for m_dram in (mU_d, mD_d, mL_d):
                    nc.sync.dma_start(out=m_dram[0:104, 260 * C:262 * C], in_=zb[0:104, :])

                # X DMA-in over d (all mask-phase reads of d are done)
                nc.sync.dma_start(out=d[:], in_=mx_d[:])

                # ---------------- backward ----------------
                if "b" not in phases:
                    continue
                def win_load(k):
                    w0 = k * WCH
                    nsl = min(WCH + 2, SE - w0)
                    tiles = {}
                    for name, m_dram in (("u", mU_d), ("d", mD_d), ("l", mL_d)):
                        w = win_pool.tile([128, (WCH + 2) * C], dt.bfloat16, tag="w" + name)
                        nc.sync.dma_start(out=w[0:104, 0:nsl * C],
                                          in_=m_dram[0:104, w0 * C:(w0 + nsl) * C])
                        tiles[name] = w
                    return tiles

                cur_k = (S - 1) // WCH
                wins = {cur_k: win_load(cur_k)}
                if cur_k - 1 >= 0:
                    wins[cur_k - 1] = win_load(cur_k - 1)
                for t in range(S - 1, 0, -1):
                    k = t // WCH
                    if k != cur_k:
                        cur_k = k
                        wins.pop(k + 2, None)
                        if k - 1 >= 0 and (k - 1) not in wins:
                            wins[k - 1] = win_load(k - 1)
                    W = wins[k]
                    lo = (t - k * WCH) * C
                    qlo, qhi = max(0, t - 256), min(3, t - 1)
                    P0, P1 = prange(qlo, qhi)
                    G = G_tiles[t % 2]
                    Sc = S_tiles[t % 2]
                    for q in (2, 1, 0):
                        nc.vector.tensor_copy(
                            G[32 * q:32 * q + 32, 65:66],
                            E[32 * (q + 1):32 * (q + 2), (t + 1) * C + 1:(t + 1) * C + 2])
                    nc.vector.tensor_tensor(
                        G[P0:P1, 1:65], E[P0:P1, (t + 1) * C + 1:(t + 1) * C + 65],
                        W["u"][P0:P1, lo + C + 1:lo + C + 65], Alu.mult)
                    nc.vector.tensor_tensor(
                        Sc[P0:P1, 1:65], E[P0:P1, (t + 1) * C + 2:(t + 1) * C + 66],
                        W["d"][P0:P1, lo + C + 2:lo + C + 66], Alu.mult)
                    nc.vector.tensor_tensor(G[P0:P1, 1:65], G[P0:P1, 1:65],
                                            Sc[P0:P1, 1:65], Alu.add)
                    if t == S - 1:
                        nc.vector.memset(G[96:128, 64:65], 1.0)
                    nc.vector.tensor_tensor_scan(
                        E[P0:P1, t * C:t * C + 66][:, ::-1],
                        W["l"][P0:P1, lo + 1:lo + 67][:, ::-1],
                        G[P0:P1, 0:66][:, ::-1], 0.0, Alu.mult, Alu.add)

                # ---------------- omega reduction ----------------
                if "x" not in phases:
                    continue
                nc.vector.tensor_tensor(E[0:104, 0:S * C], E[0:104, 0:S * C],
                                        d[0:104, 0:S * C], Alu.mult)
                nc.vector.tensor_tensor(E[0:104, 0:S * C], E[0:104, 0:S * C],
                                        d[0:104, 0:S * C], Alu.mult)
                nc.vector.tensor_reduce(
                    pt_t[0:104, 0:1],
                    E[0:104, 0:S * C].rearrange("p (s c) -> p s c", c=C),
                    mybir.AxisListType.XY, Alu.add)
                nc.sync.dma_start(out=pt_d[:], in_=pt_t[:])

    nc.compile()
    return nc


def _host_inputs(y_true, y_pred):
    """Per-core input dict list. y_true/y_pred: [B, N] f32."""
    in_maps = []
    rows = np.arange(S)  # slot t
    for core in range(NCORES):
        dT = np.zeros((128, S), f32)
        dO = np.zeros((128, 64), f32)
        mx = np.zeros((128, S, C), bf16)
        for q in range(Q):
            i = rows - q  # row index per slot
            valid = (i >= 1) & (i <= N)
            for s in range(SPC):
                b = core * SPC + s
                p = 32 * q + s
                dT[p, valid] = y_true[b, i[valid] - 1]
                dO[p, :] = y_pred[b, q * 64:(q + 1) * 64]
                m = np.arange(1, 65)[None, :]
                X = (i[:, None] - (q * 64 + m)).astype(f32)
                X[~valid, :] = 0.0
                mx[p, :, 1:] = X.astype(bf16)
        in_maps.append({"dT": dT, "dO": dO, "mx": mx.reshape(128, S * C)})
    return in_maps


def kernel(y_pred, y_true):
    yp = np.asarray(y_pred, dtype=f32).reshape(B, N)
    yt = np.asarray(y_true, dtype=f32).reshape(B, N)
    if "nc" not in _cache:
        _cache["nc"] = _build()
    nc = _cache["nc"]
    from concourse.bass_utils import run_bass_kernel_spmd
    in_maps = _host_inputs(yt, yp)
    res = run_bass_kernel_spmd(nc, in_maps, core_ids=list(range(NCORES)))
    shape_vals = []
    temp_sum = 0.0
    for core in range(NCORES):
        r = res.results[core]
        shape_vals.append(r["ps"][:, 0])
        pt = r["pt"][:, 0]
        for q in range(Q):
            for s in range(SPC):
                temp_sum += float(pt[32 * q + s])
    loss_shape = float(np.mean(np.concatenate(shape_vals)))
    loss_temporal = temp_sum / B / (N * N)
    loss = ALPHA * loss_shape + (1.0 - ALPHA) * loss_temporal
    return np.array(loss, dtype=f32)

